# revision 11
# baseline (speedup 1.0000x reference)
"""Trainium2 Bass kernel for DensePairwiseRelaxedWordMoverSimilarity.

Shapes (hardcoded): x1 (64,128,512) f32, mask1 (64,128) bool,
                    x2 (64,128,512) f32, mask2 (64,128) bool -> out (64,64) f32.

Strategy: data-parallel over B1 across 8 cores; each core holds its 8-row
x1 slab plus the full x2 and computes an (8, 64) output slab.

Per-core device program (SPMD, identical on all cores):
  - inputs arrive host-transposed to d-major layout (a pure permutation):
      xT[d, :1024]      = x1_slab[a,s,d]   (cols a*128+s)
      xT[d, 1024:9216]  = x2[b,t,d]        (cols 1024 + b*128+t)
  - normalize on-device: ACT squares -> ones-matmul partition sums (PSUM
    accumulation over the 4 d-chunks) -> ACT rsqrt -> mask-zeroed scale
    (DVE), writing float32r-rounded data in place.
  - main loop over (a, bgroup-of-4-b): 4 f32r matmuls (N=512) accumulate
    S = n1[a] @ n2[bg]^T in PSUM; ACT copies S to SBUF (f32r); DVE
    segmented reduce_max gives sim1 (max over t); 4 PE transposes give
    S^T in PSUM; DVE segmented reduce_max gives sim2 (max over s).
    Masked rows were zeroed, so masked entries hold 0 and lose the max
    (all-valid sims being negative is probability ~2^-64 per row; verified
    against the reference by test.py).
  - means: (m1+m2)/2 via two tiny weighted-sum matmuls per a with host
    precomputed weights w = 0.5*mask/len, accumulated in one PSUM row.
"""

import numpy as np

import concourse.bass as bass
import concourse.bacc as bacc
import concourse.mybir as mybir
from concourse import tile
from concourse.bass_utils import run_bass_kernel_spmd

F32 = mybir.dt.float32
F32R = mybir.dt.float32r
AX = mybir.AxisListType

B1, S1, B2, S2, D = 64, 128, 64, 128, 512
NCORES = 8
A = B1 // NCORES          # 8 a-rows per core
KC = D // 128             # 4 contraction chunks
X1W = A * S1              # 1024 x1 columns in xT
X2W = B2 * S2             # 8192 x2 columns in xT
XW = X1W + X2W            # 9216
NBG = B2 // 4             # 16 b-groups of 4 b's (512 t-columns)

_CACHE = {}


def _build():
    nc = bacc.Bacc(None, target_bir_lowering=False, debug=False)

    xT = nc.declare_dram_parameter("xT", [D, XW], F32, isOutput=False)
    identp = nc.declare_dram_parameter("identp", [128, 128], F32, isOutput=False)
    onesp = nc.declare_dram_parameter("onesp", [128, 128], F32, isOutput=False)
    # consts: cols 0:8 w1T, 8:72 w2T, 72 ones-column
    consts = nc.declare_dram_parameter("consts", [128, 73], F32, isOutput=False)
    m2row = nc.declare_dram_parameter("m2row", [1, X2W], F32, isOutput=False)
    m1row = nc.declare_dram_parameter("m1row", [1, X1W], F32, isOutput=False)
    out = nc.declare_dram_parameter("out", [1, A * B2], F32, isOutput=True)

    CG = 2048             # normalize column-group width

    with tile.TileContext(nc) as tc:
        with (
            tc.tile_pool(name="xts", bufs=1) as xts_pool,
            tc.tile_pool(name="cst", bufs=1) as cst_pool,
            tc.tile_pool(name="coll", bufs=2) as coll_pool,
            tc.tile_pool(name="normtmp", bufs=2) as norm_pool,
            tc.tile_pool(name="cpool", bufs=3) as cpool,
            tc.tile_pool(name="psB", bufs=2, space="PSUM") as psB,
            tc.tile_pool(name="psS", bufs=3, space="PSUM") as psS,
            tc.tile_pool(name="psT", bufs=2, space="PSUM") as psT,
            tc.tile_pool(name="psm", bufs=1, space="PSUM") as psm,
        ):
            # ---- stage A: loads ----
            xts = xts_pool.tile([128, KC, XW], F32R, tag="xts")
            nc.gpsimd.dma_start(
                xts[:], xT.ap().rearrange("(k p) m -> p k m", p=128)
            )
            ident = cst_pool.tile([128, 128], F32R, tag="ident")
            nc.gpsimd.dma_start(ident[:], identp.ap())
            ones128 = cst_pool.tile([128, 128], F32R, tag="ones128")
            nc.gpsimd.dma_start(ones128[:], onesp.ap())
            csts = cst_pool.tile([128, 73], F32, tag="consts")
            nc.sync.dma_start(csts[:], consts.ap())
            w1T = csts[:, 0:8]
            w2T = csts[:, 8:72]
            onescol = csts[:, 72:73]

            # ---- stage B: normalize all 9216 columns ----
            # 18 column-groups of 512; norms^2 accumulate as ROWS of one
            # PSUM bank: pn[g, :] = sum_d xts[d, g*512:(g+1)*512]^2 via
            # M=1 ones-matmuls with per-group partition offset.
            NG = XW // 512  # 18
            # masks as [18, 512] (m1row cols 0:1024 -> rows 0:2, m2row -> 2:18)
            mrows = norm_pool.tile([NG, 512], F32, tag="mrows")
            nc.sync.dma_start(
                mrows[0 : X1W // 512, :],
                m1row.ap().rearrange("x (g n) -> (x g) n", n=512),
            )
            nc.sync.dma_start(
                mrows[X1W // 512 : NG, :],
                m2row.ap().rearrange("x (g n) -> (x g) n", n=512),
            )
            # squares in [128, 2048] chunks (x1: one 1024 chunk); each
            # 512-col group's norms^2 accumulates in a [1,512] PSUM tile
            # (M=1 ones-matmul) and is copied out to row g of norms2_sb.
            norms2 = norm_pool.tile([NG, 512], F32, tag="norms2")
            sqgroups = [(0, X1W)] + [
                (X1W + g * CG, CG) for g in range(X2W // CG)
            ]
            for g in range(NG):
                pn = psB.tile([128, 512], F32, tag="bstage")
                for k in range(KC):
                    sq = norm_pool.tile([128, 512], F32, tag="sq")
                    nc.scalar.activation(
                        sq[:].bitcast(F32R),
                        xts[:, k, g * 512 : (g + 1) * 512].bitcast(F32),
                        mybir.ActivationFunctionType.Square,
                    )
                    nc.tensor.matmul(
                        pn[0:1, :],
                        ones128[:, 0:1],
                        sq[:].bitcast(F32R),
                        start=(k == 0),
                        stop=(k == KC - 1),
                    )
                # sqrt on the [1,512] row, then DMA-shuttle it to row g of
                # norms2 (engine APs need 32-aligned partition bases; DMA
                # is the only unaligned-partition mover).
                stg = norm_pool.tile([1, 512], F32, tag="stg")
                nc.scalar.activation(
                    stg[:], pn[0:1, :], mybir.ActivationFunctionType.Sqrt,
                )
                nc.sync.dma_start(norms2[g : g + 1, :], stg[:])
            # bulk inv_norm rows: reciprocal + mask-zero on [18, 512]
            inv_sb = norm_pool.tile([NG, 512], F32, tag="inv_sb")
            nc.vector.reciprocal(inv_sb[:], norms2[:])
            nc.vector.tensor_mul(inv_sb[:], inv_sb[:], mrows[:])
            # broadcast each inv row to [128, 512] via K=1 matmul, then scale
            for g in range(NG):
                stg2 = norm_pool.tile([1, 512], F32R, tag="stg2")
                nc.gpsimd.dma_start(stg2[:], inv_sb[g : g + 1, :])
                pinv = psB.tile([128, 512], F32, tag="bstage")
                nc.tensor.matmul(
                    pinv[:],
                    ones128[0:1, :],
                    stg2[:],
                    start=True,
                    stop=True,
                )
                for k in range(KC):
                    nc.vector.tensor_mul(
                        xts[:, k, g * 512 : (g + 1) * 512],
                        xts[:, k, g * 512 : (g + 1) * 512].bitcast(F32),
                        pinv[:],
                    )

            # ---- stage C: pairwise sim + maxes ----
            mps = psm.tile([1, A * B2], F32, tag="mps")
            for a in range(A):
                sim1 = coll_pool.tile([128, B2], F32, tag="sim1")
                sim2 = coll_pool.tile([128, B2], F32, tag="sim2")
                for bg in range(NBG):
                    S = psS.tile([128, 512], F32, tag="S")
                    for k in range(KC):
                        nc.tensor.matmul(
                            S[:],
                            xts[:, k, a * 128 : (a + 1) * 128],
                            xts[:, k, X1W + bg * 512 : X1W + (bg + 1) * 512],
                            start=(k == 0),
                            stop=(k == KC - 1),
                        )
                    C = cpool.tile([128, 512], F32R, tag="C")
                    nc.scalar.copy(C[:], S[:])
                    nc.vector.reduce_max(
                        sim1[:, bg * 4 : (bg + 1) * 4],
                        C[:].bitcast(F32).rearrange("p (b t) -> p b t", b=4),
                        axis=AX.X,
                    )
                    T = psT.tile([128, 512], F32, tag="T")
                    for j in range(4):
                        nc.tensor.transpose(
                            T[:, j * 128 : (j + 1) * 128].bitcast(F32R),
                            C[:, j * 128 : (j + 1) * 128],
                            ident[:],
                        )
                    nc.vector.reduce_max(
                        sim2[:, bg * 4 : (bg + 1) * 4],
                        T[:].rearrange("p (b s) -> p b s", b=4),
                        axis=AX.X,
                    )
                # weighted means: mps[0, a*64:(a+1)*64] = w1.T@sim1 + 1.T@(sim2*w2)
                sim2w = coll_pool.tile([128, B2], F32, tag="sim2w")
                nc.vector.tensor_mul(sim2w[:], sim2[:], w2T)
                nc.tensor.matmul(
                    mps[:, a * B2 : (a + 1) * B2],
                    w1T[:, a : a + 1],
                    sim1[:],
                    start=True,
                    stop=False,
                )
                nc.tensor.matmul(
                    mps[:, a * B2 : (a + 1) * B2],
                    onescol,
                    sim2w[:],
                    start=False,
                    stop=True,
                )
            outs = cst_pool.tile([1, A * B2], F32, tag="outs")
            nc.scalar.copy(outs[:], mps[:])
            nc.sync.dma_start(out.ap(), outs[:])
    nc.finalize()
    return nc


def _prep_inputs(x1, mask1, x2, mask2):
    """Host-side marshaling: layout transposes, mask->weight conversion."""
    x1 = np.asarray(x1, dtype=np.float32)
    x2 = np.asarray(x2, dtype=np.float32)
    m1 = np.asarray(mask1).astype(bool)
    m2 = np.asarray(mask2).astype(bool)

    ident = np.eye(128, dtype=np.float32)
    ones128 = np.ones((128, 128), dtype=np.float32)
    m2row = np.ascontiguousarray(
        m2.astype(np.float32).reshape(1, X2W)
    )
    x2T = np.ascontiguousarray(x2.reshape(X2W, D).T)  # [512, 8192]

    len1 = np.maximum(m1.sum(axis=1), 1).astype(np.float32)  # [64]
    len2 = np.maximum(m2.sum(axis=1), 1).astype(np.float32)  # [64]
    w1 = m1.astype(np.float32) * (0.5 / len1)[:, None]  # [64, 128]
    w2 = m2.astype(np.float32) * (0.5 / len2)[:, None]  # [64, 128]
    w2T = np.ascontiguousarray(w2.T)  # [128, 64]

    in_maps = []
    for c in range(NCORES):
        sl = slice(c * A, (c + 1) * A)
        x1T = np.ascontiguousarray(x1[sl].reshape(X1W, D).T)  # [512, 1024]
        xT = np.ascontiguousarray(np.concatenate([x1T, x2T], axis=1))
        m1row = np.ascontiguousarray(
            m1[sl].astype(np.float32).reshape(1, X1W)
        )
        w1T = np.ascontiguousarray(w1[sl].T)  # [128, 8]
        consts = np.concatenate(
            [w1T, w2T, np.ones((128, 1), np.float32)], axis=1
        )  # [128, 73]
        in_maps.append(
            {
                "xT": xT,
                "identp": ident,
                "onesp": ones128,
                "consts": np.ascontiguousarray(consts),
                "m2row": m2row,
                "m1row": m1row,
            }
        )
    return in_maps


def kernel(x1, mask1, x2, mask2):
    if "nc" not in _CACHE:
        _CACHE["nc"] = _build()
    nc = _CACHE["nc"]
    in_maps = _prep_inputs(x1, mask1, x2, mask2)
    res = run_bass_kernel_spmd(nc, in_maps, list(range(NCORES)))
    rows = [res.results[c]["out"].reshape(A, B2) for c in range(NCORES)]
    return np.ascontiguousarray(np.concatenate(rows, axis=0).astype(np.float32))
